# revision 24
# baseline (speedup 1.0000x reference)
"""Trainium2 Bass kernel for nn_MeshLoss2D (chamfer min-distance mesh loss) v5.

Math: refine (B,3,32,32) mesh bilinearly x3 to (B,3,94,94); per pc point
(B,3,4096) min squared distance to refined mesh; mean.
Sharding: 8 cores = (batch, pc half): 2048 pc x 8836 mesh points per core
(mesh padded to 8960 with a +60000 sentinel norm).

Three-engine reduce per core:
- NORMAL orientation (mesh cols [0, FLIP0), per 128-pc tile):
    V quads -> DVE tensor_scalar min-accum direct from PSUM
    A quads -> ACT evac to fp16 ev buffer; ONE DVE 4x-packed accum per tile
- FLIPPED orientation (mesh cols [FLIP0, 8960) in 128-mesh-partition blocks
  x 2048 pc): ACT evac with scale=-1, Pool cross-lane (axis=C) MAX per
  block -> [1,2048] partial -> DMA to DRAM; host takes -max over blocks.
  (Cross-lane reduce only supports max, hence negation. Pool cannot read
  PSUM, hence the ACT evac.)
Flip-block halves are interleaved between normal quads to smooth engine
queues (uniform 4/3 per-tile flip schedule).
Host: final = mean(min(normal_min, flip_min) + ||p||^2).
"""

import os
import sys

for _p in ("/opt/trn_rl_repo", "/opt/trn_rl_repo/concourse"):
    if _p not in sys.path:
        sys.path.insert(0, _p)

import numpy as np

B, C, H, W = 4, 3, 32, 32
FACTOR = 3
OH = (H - 1) * FACTOR + 1        # 94
N_MESH = OH * OH                 # 8836
N_PAD = 8960                     # 70 blocks of 128
M_TOTAL = 4096
N_CORES = 8
M_CORE = M_TOTAL * B // N_CORES  # 2048
PC_TILES = M_CORE // 128         # 16
PAD_BIG = 60000.0

_BUILT = {}
LAST_RESULTS = None

DEFAULT_CFG = dict(
    flip0=5376,                   # 28 flip blocks
    # normal quads over [0, flip0): (width, route)
    quads=[(1024, "A"), (1024, "V"), (1024, "V"), (384, "A"),
           (896, "V"), (1024, "V")],
    # flip halves consumed per tile (sum must be 2*n_fblk)
    fhalves=[4, 3, 4, 3, 4, 3, 4, 3, 4, 3, 4, 3, 4, 3, 4, 3],
    psum_bufs=4, fpsum_bufs=0, ev_bufs=4, fev_bufs=4,
)

_PADV = np.zeros((12, N_PAD - N_MESH), dtype=np.float16)
_PADV[3, :] = np.float16(PAD_BIG)


def _interp_matrix():
    ys = np.arange(OH, dtype=np.float32) / np.float32(FACTOR)
    y0 = np.clip(np.floor(ys).astype(np.int64), 0, H - 2)
    wy = ys - y0.astype(np.float32)
    R = np.zeros((OH, H), dtype=np.float32)
    R[np.arange(OH), y0] = np.float32(1.0) - wy
    R[np.arange(OH), y0 + 1] += wy
    return R


def _build_kernel(cfg=None):
    from concourse import bacc, mybir
    import concourse.tile as tile

    if cfg is None:
        cfg = DEFAULT_CFG
    flip0 = cfg["flip0"]
    quads = cfg["quads"]
    n_fblk = (N_PAD - flip0) // 128
    fhalves = cfg["fhalves"]
    assert sum(fhalves) == 2 * n_fblk
    assert sum(w for w, _ in quads) == flip0

    f32 = mybir.dt.float32
    f16 = mybir.dt.float16
    MIN = mybir.AluOpType.min
    MAX = mybir.AluOpType.max
    MULT = mybir.AluOpType.mult
    SUB = mybir.AluOpType.subtract
    ADD = mybir.AluOpType.add

    nc = bacc.Bacc("TRN2", target_bir_lowering=False, debug=False,
                   enable_asserts=False, num_devices=N_CORES)

    pre = nc.dram_tensor("pre", (C * H, OH + C * W), f32, kind="ExternalInput").ap()
    pcs = nc.dram_tensor("a_aug", (12, M_CORE), f16, kind="ExternalInput").ap()
    padv = nc.dram_tensor("padv", (12, N_PAD - N_MESH), f16, kind="ExternalInput").ap()
    out_min = nc.dram_tensor("minaug", (128, PC_TILES), f32, kind="ExternalOutput").ap()
    out_flip = nc.dram_tensor("flipmax", (n_fblk, M_CORE), f16, kind="ExternalOutput").ap()

    nA = sum(1 for (_, r) in quads if r == "A")
    nV = sum(1 for (_, r) in quads if r == "V")
    a_cols = sum(w for (w, r) in quads if r == "A")
    gv = nV + (1 if nA else 0)

    with tile.TileContext(nc) as tc:
        with tc.tile_pool(name="const", bufs=1) as cpool, \
             tc.tile_pool(name="dram", bufs=1, space="DRAM") as dpool:

            # ---------------- load inputs ----------------
            pre_sb = cpool.tile([C * H, OH + C * W], f32)
            nc.sync.dma_start(out=pre_sb[:], in_=pre)
            rm_sb = pre_sb[:, 0:OH]                  # R^T replicated per quadrant
            g_sb = pre_sb[0:H, OH:OH + C * W]        # [32y, 96(c,x)]
            aaug = cpool.tile([12, M_CORE], f16)
            nc.sync.dma_start(out=aaug[:], in_=pcs)
            baug = cpool.tile([12, N_PAD], f16)
            nc.sync.dma_start(out=baug[:, N_MESH:N_PAD], in_=padv)

            # ------------- mesh refine on PE (fp32, 1+3 matmuls) -------------
            mstage = cpool.tile([OH, C * OH], f32)
            a_sb = cpool.tile([C * H, OH], f32)      # [(c,x)=96, 94]
            with tc.tile_pool(name="rpsum", bufs=2, space="PSUM") as rpool:
                pA = rpool.tile([C * H, OH], f32, name="pA")
                nc.tensor.matmul(out=pA[:], lhsT=g_sb, rhs=rm_sb[0:H, 0:OH],
                                 start=True, stop=True)
                nc.vector.tensor_copy(a_sb[:], pA[:])
                for c in range(C):
                    pB = rpool.tile([OH, OH], f32, name="pB")
                    nc.tensor.matmul(
                        out=pB[:], lhsT=a_sb[c * H:(c + 1) * H, :],
                        rhs=rm_sb[c * H:(c + 1) * H, 0:OH], start=True, stop=True)
                    nc.vector.tensor_copy(mstage[:, c * OH:(c + 1) * OH], pB[:])

            # -------- ||m||^2 and fp16 hi/lo staging (ACT + DVE split) -------
            sqs = cpool.tile([OH, C * OH], f32)
            sq01 = cpool.tile([OH, OH], f32)
            sqsum = cpool.tile([OH, OH], f32)
            bhi = cpool.tile([OH, 4 * OH], f16)
            blo = cpool.tile([OH, 4 * OH], f16)
            for c in range(C):                       # squares on ACT
                nc.scalar.square(sqs[:, c * OH:(c + 1) * OH],
                                 mstage[:, c * OH:(c + 1) * OH])
            nc.vector.tensor_tensor(out=sq01[:], in0=sqs[:, 0:OH],
                                    in1=sqs[:, OH:2 * OH], op=ADD)
            nc.vector.tensor_tensor(out=sqsum[:], in0=sq01[:],
                                    in1=sqs[:, 2 * OH:3 * OH], op=ADD)
            for c in range(C):
                mc = mstage[:, c * OH:(c + 1) * OH]
                hc = bhi[:, c * OH:(c + 1) * OH]
                lc = blo[:, c * OH:(c + 1) * OH]
                nc.scalar.mul(hc, mc, -2.0)          # hi on ACT
                nc.vector.scalar_tensor_tensor(      # lo residual on DVE
                    out=lc, in0=mc, scalar=-2.0, in1=hc, op0=MULT, op1=SUB)
            hs = bhi[:, 3 * OH:4 * OH]
            ls = blo[:, 3 * OH:4 * OH]
            nc.scalar.copy(hs, sqsum[:])
            nc.vector.scalar_tensor_tensor(
                out=ls, in0=sqsum[:], scalar=1.0, in1=hs, op0=MULT, op1=SUB)

            # ---------------- flatten via DRAM roundtrip + pad ---------------
            dhi = dpool.tile([4, N_MESH], f16)
            dlo = dpool.tile([4, N_MESH], f16)
            nc.sync.dma_start(
                out=dhi[:].rearrange("c (h w) -> h c w", h=OH),
                in_=bhi[:].rearrange("h (c w) -> h c w", c=4))
            nc.sync.dma_start(
                out=dlo[:].rearrange("c (h w) -> h c w", h=OH),
                in_=blo[:].rearrange("h (c w) -> h c w", c=4))
            nc.sync.dma_start(out=baug[0:4, 0:N_MESH], in_=dhi[:])
            nc.sync.dma_start(out=baug[4:8, 0:N_MESH], in_=dlo[:])
            nc.sync.dma_start(out=baug[8:12, 0:N_MESH], in_=dhi[:])

            # ---------------- main loop ----------------
            results = cpool.tile([128, PC_TILES], f32)
            trash_v = cpool.tile([128, 1024], f16)
            trash_a = cpool.tile([128, max(a_cols, 1)], f16)
            accs_v = cpool.tile([128, PC_TILES * gv], f32)

            with tc.tile_pool(name="mpsum", bufs=cfg["psum_bufs"], space="PSUM") as mpool, \
                 tc.tile_pool(name="evac", bufs=cfg["ev_bufs"]) as epool, \
                 tc.tile_pool(name="fevac", bufs=cfg["fev_bufs"]) as fpool:
                fh = 0            # global flip-half counter
                fev = None
                fq_cur = None

                def emit_flip_half():
                    nonlocal fh, fev
                    j, h = fh // 2, fh % 2
                    mb = flip0 + j * 128
                    if h == 0:
                        fev = fpool.tile([128, M_CORE], f16, name="fev")
                    fq = mpool.tile([128, 1024], f32, name="pd")
                    for s in range(2):
                        nc.tensor.matmul(
                            out=fq[:, s * 512:(s + 1) * 512],
                            lhsT=baug[:, mb:mb + 128],
                            rhs=aaug[:, h * 1024 + s * 512:h * 1024 + (s + 1) * 512],
                            start=True, stop=True)
                    nc.scalar.mul(fev[:, h * 1024:(h + 1) * 1024], fq[:], -1.0)
                    if h == 1:
                        fpart = fpool.tile([1, M_CORE], f16, name="fpart")
                        nc.gpsimd.tensor_reduce(
                            fpart[:], fev[:], axis=mybir.AxisListType.C, op=MAX)
                        nc.sync.dma_start(out=out_flip[j:j + 1, :], in_=fpart[:])
                    fh += 1

                for t in range(PC_TILES):
                    tquads = quads
                    lh = aaug[:, t * 128:(t + 1) * 128]
                    ev = epool.tile([128, max(a_cols, 1)], f16, name="ev")
                    ev_fill = 0
                    jv = 0
                    ja = 0
                    todo_f = fhalves[t]
                    nq = len(quads)
                    c0 = 0

                    for qi, (w, r) in enumerate(tquads):
                        pd = mpool.tile([128, 1024], f32, name="pd")
                        fd = 0
                        while fd < w:
                            cw = min(512, w - fd)
                            nc.tensor.matmul(
                                out=pd[:, fd:fd + cw], lhsT=lh,
                                rhs=baug[:, c0 + fd:c0 + fd + cw],
                                start=True, stop=True)
                            fd += cw
                        if r == "V":
                            col = t * gv + jv
                            nc.vector.tensor_scalar(
                                out=trash_v[:, 0:w], in0=pd[:, 0:w],
                                scalar1=1e30, scalar2=None, op0=MIN, op1=MIN,
                                accum_out=accs_v[:, col:col + 1])
                            jv += 1
                        else:
                            nc.scalar.copy(ev[:, ev_fill:ev_fill + w], pd[:, 0:w])
                            ev_fill += w
                        c0 += w
                        # interleave flip halves across the tile
                        want = (todo_f * (qi + 1)) // nq
                        while fh - (sum(fhalves[:t])) < want:
                            emit_flip_half()
                    while fh < sum(fhalves[:t + 1]):
                        emit_flip_half()
                    if a_cols:
                        nc.vector.tensor_scalar(
                            out=trash_a[:, 0:ev_fill], in0=ev[:, 0:ev_fill],
                            scalar1=1e30, scalar2=None, op0=MIN, op1=MIN,
                            accum_out=accs_v[:, t * gv + nV:t * gv + nV + 1])
                    if t == PC_TILES // 2 - 1:
                        nc.vector.tensor_reduce(
                            results[:, 0:PC_TILES // 2],
                            accs_v[:, 0:gv * PC_TILES // 2].rearrange(
                                "p (t g) -> p t g", t=PC_TILES // 2),
                            axis=mybir.AxisListType.X, op=MIN)
                        nc.sync.dma_start(out=out_min[:, 0:PC_TILES // 2],
                                          in_=results[:, 0:PC_TILES // 2])

                nc.vector.tensor_reduce(
                    results[:, PC_TILES // 2:],
                    accs_v[:, gv * PC_TILES // 2:].rearrange(
                        "p (t g) -> p t g", t=PC_TILES // 2),
                    axis=mybir.AxisListType.X, op=MIN)

            nc.sync.dma_start(out=out_min[:, PC_TILES // 2:],
                              in_=results[:, PC_TILES // 2:])

    nc.compile()
    return nc


def _get_nc():
    if "nc" not in _BUILT:
        _BUILT["nc"] = _build_kernel()
    return _BUILT["nc"]


def _make_a_aug(pc_slice: np.ndarray) -> np.ndarray:
    m = pc_slice.shape[1]
    hi = pc_slice.astype(np.float16)
    lo = (pc_slice - hi.astype(np.float32)).astype(np.float16)
    a = np.zeros((12, m), dtype=np.float16)
    a[0:3] = hi
    a[3] = np.float16(1.0)
    a[4:7] = hi
    a[7] = np.float16(1.0)
    a[8:11] = lo
    a[11] = np.float16(0.0)
    return a


def kernel(network_mesh: np.ndarray, pc: np.ndarray) -> np.ndarray:
    global LAST_RESULTS
    from concourse.bass_utils import run_bass_kernel_spmd

    network_mesh = np.ascontiguousarray(network_mesh, dtype=np.float32)
    pc = np.ascontiguousarray(pc, dtype=np.float32)

    nc = _get_nc()
    rmat3 = np.tile(_interp_matrix().T, (C, 1))          # [96, 94]

    in_maps = []
    for core in range(N_CORES):
        b, h = core // 2, core % 2
        pre = np.zeros((C * H, OH + C * W), dtype=np.float32)
        pre[:, 0:OH] = rmat3
        pre[0:H, OH:] = network_mesh[b].transpose(1, 0, 2).reshape(H, C * W)
        in_maps.append({
            "pre": pre,
            "a_aug": _make_a_aug(pc[b, :, h * M_CORE:(h + 1) * M_CORE]),
            "padv": _PADV,
        })

    res = run_bass_kernel_spmd(nc, in_maps, core_ids=list(range(N_CORES)))
    LAST_RESULTS = res

    pnorm = np.sum(pc * pc, axis=1)
    vals = []
    for core in range(N_CORES):
        b, h = core // 2, core % 2
        v_norm = res.results[core]["minaug"].T.reshape(M_CORE)
        v_flip = -res.results[core]["flipmax"].astype(np.float32).max(axis=0)
        v = np.minimum(v_norm, v_flip)
        vals.append(v + pnorm[b, h * M_CORE:(h + 1) * M_CORE])
    dist2 = np.concatenate(vals)
    return np.array(np.mean(dist2, dtype=np.float32), dtype=np.float32)


# revision 25
# speedup vs baseline: 1.0115x; 1.0115x over previous
"""Trainium2 Bass kernel for nn_MeshLoss2D (chamfer min-distance mesh loss) v5.

Math: refine (B,3,32,32) mesh bilinearly x3 to (B,3,94,94); per pc point
(B,3,4096) min squared distance to refined mesh; mean.
Sharding: 8 cores = (batch, pc half): 2048 pc x 8836 mesh points per core
(mesh padded to 8960 with a +60000 sentinel norm).

Three-engine reduce per core:
- NORMAL orientation (mesh cols [0, FLIP0), per 128-pc tile):
    V quads -> DVE tensor_scalar min-accum direct from PSUM
    A quads -> ACT evac to fp16 ev buffer; ONE DVE 4x-packed accum per tile
- FLIPPED orientation (mesh cols [FLIP0, 8960) in 128-mesh-partition blocks
  x 2048 pc): ACT evac with scale=-1, Pool cross-lane (axis=C) MAX per
  block -> [1,2048] partial -> DMA to DRAM; host takes -max over blocks.
  (Cross-lane reduce only supports max, hence negation. Pool cannot read
  PSUM, hence the ACT evac.)
Flip-block halves are interleaved between normal quads to smooth engine
queues (uniform 4/3 per-tile flip schedule).
Host: final = mean(min(normal_min, flip_min) + ||p||^2).
"""

import os
import sys

for _p in ("/opt/trn_rl_repo", "/opt/trn_rl_repo/concourse"):
    if _p not in sys.path:
        sys.path.insert(0, _p)

import numpy as np

B, C, H, W = 4, 3, 32, 32
FACTOR = 3
OH = (H - 1) * FACTOR + 1        # 94
N_MESH = OH * OH                 # 8836
N_PAD = 8960                     # 70 blocks of 128
M_TOTAL = 4096
N_CORES = 8
M_CORE = M_TOTAL * B // N_CORES  # 2048
PC_TILES = M_CORE // 128         # 16
PAD_BIG = 60000.0

_BUILT = {}
LAST_RESULTS = None

DEFAULT_CFG = dict(
    flip0=5376,                   # 28 flip blocks
    # normal quads over [0, flip0): (width, route)
    quads=[(1024, "A"), (1024, "V"), (1024, "V"), (512, "A"),
           (768, "V"), (1024, "V")],
    # flip halves consumed per tile (sum must be 2*n_fblk)
    fhalves=[4, 3, 4, 3, 4, 3, 4, 3, 4, 3, 4, 3, 4, 3, 4, 3],
    psum_bufs=4, fpsum_bufs=0, ev_bufs=4, fev_bufs=4,
)

_PADV = np.zeros((12, N_PAD - N_MESH), dtype=np.float16)
_PADV[3, :] = np.float16(PAD_BIG)


def _interp_matrix():
    ys = np.arange(OH, dtype=np.float32) / np.float32(FACTOR)
    y0 = np.clip(np.floor(ys).astype(np.int64), 0, H - 2)
    wy = ys - y0.astype(np.float32)
    R = np.zeros((OH, H), dtype=np.float32)
    R[np.arange(OH), y0] = np.float32(1.0) - wy
    R[np.arange(OH), y0 + 1] += wy
    return R


def _build_kernel(cfg=None):
    from concourse import bacc, mybir
    import concourse.tile as tile

    if cfg is None:
        cfg = DEFAULT_CFG
    flip0 = cfg["flip0"]
    quads = cfg["quads"]
    n_fblk = (N_PAD - flip0) // 128
    fhalves = cfg["fhalves"]
    assert sum(fhalves) == 2 * n_fblk
    assert sum(w for w, _ in quads) == flip0

    f32 = mybir.dt.float32
    f16 = mybir.dt.float16
    MIN = mybir.AluOpType.min
    MAX = mybir.AluOpType.max
    MULT = mybir.AluOpType.mult
    SUB = mybir.AluOpType.subtract
    ADD = mybir.AluOpType.add

    nc = bacc.Bacc("TRN2", target_bir_lowering=False, debug=False,
                   enable_asserts=False, num_devices=N_CORES)

    pre = nc.dram_tensor("pre", (C * H, OH + C * W), f32, kind="ExternalInput").ap()
    pcs = nc.dram_tensor("a_aug", (12, M_CORE), f16, kind="ExternalInput").ap()
    padv = nc.dram_tensor("padv", (12, N_PAD - N_MESH), f16, kind="ExternalInput").ap()
    out_min = nc.dram_tensor("minaug", (128, PC_TILES), f32, kind="ExternalOutput").ap()
    out_flip = nc.dram_tensor("flipmax", (n_fblk, M_CORE), f16, kind="ExternalOutput").ap()

    nA = sum(1 for (_, r) in quads if r == "A")
    nV = sum(1 for (_, r) in quads if r == "V")
    a_cols = sum(w for (w, r) in quads if r == "A")
    gv = nV + (1 if nA else 0)

    with tile.TileContext(nc) as tc:
        with tc.tile_pool(name="const", bufs=1) as cpool, \
             tc.tile_pool(name="dram", bufs=1, space="DRAM") as dpool:

            # ---------------- load inputs ----------------
            pre_sb = cpool.tile([C * H, OH + C * W], f32)
            nc.sync.dma_start(out=pre_sb[:], in_=pre)
            rm_sb = pre_sb[:, 0:OH]                  # R^T replicated per quadrant
            g_sb = pre_sb[0:H, OH:OH + C * W]        # [32y, 96(c,x)]
            aaug = cpool.tile([12, M_CORE], f16)
            nc.sync.dma_start(out=aaug[:], in_=pcs)
            baug = cpool.tile([12, N_PAD], f16)
            nc.sync.dma_start(out=baug[:, N_MESH:N_PAD], in_=padv)

            # ------------- mesh refine on PE (fp32, 1+3 matmuls) -------------
            mstage = cpool.tile([OH, C * OH], f32)
            a_sb = cpool.tile([C * H, OH], f32)      # [(c,x)=96, 94]
            with tc.tile_pool(name="rpsum", bufs=2, space="PSUM") as rpool:
                pA = rpool.tile([C * H, OH], f32, name="pA")
                nc.tensor.matmul(out=pA[:], lhsT=g_sb, rhs=rm_sb[0:H, 0:OH],
                                 start=True, stop=True)
                nc.vector.tensor_copy(a_sb[:], pA[:])
                for c in range(C):
                    pB = rpool.tile([OH, OH], f32, name="pB")
                    nc.tensor.matmul(
                        out=pB[:], lhsT=a_sb[c * H:(c + 1) * H, :],
                        rhs=rm_sb[c * H:(c + 1) * H, 0:OH], start=True, stop=True)
                    nc.vector.tensor_copy(mstage[:, c * OH:(c + 1) * OH], pB[:])

            # -------- ||m||^2 and fp16 hi/lo staging (ACT + DVE split) -------
            sqs = cpool.tile([OH, C * OH], f32)
            sq01 = cpool.tile([OH, OH], f32)
            sqsum = cpool.tile([OH, OH], f32)
            bhi = cpool.tile([OH, 4 * OH], f16)
            blo = cpool.tile([OH, 4 * OH], f16)
            for c in range(C):                       # squares on ACT
                nc.scalar.square(sqs[:, c * OH:(c + 1) * OH],
                                 mstage[:, c * OH:(c + 1) * OH])
            nc.vector.tensor_tensor(out=sq01[:], in0=sqs[:, 0:OH],
                                    in1=sqs[:, OH:2 * OH], op=ADD)
            nc.vector.tensor_tensor(out=sqsum[:], in0=sq01[:],
                                    in1=sqs[:, 2 * OH:3 * OH], op=ADD)
            for c in range(C):
                mc = mstage[:, c * OH:(c + 1) * OH]
                hc = bhi[:, c * OH:(c + 1) * OH]
                lc = blo[:, c * OH:(c + 1) * OH]
                nc.scalar.mul(hc, mc, -2.0)          # hi on ACT
                nc.vector.scalar_tensor_tensor(      # lo residual on DVE
                    out=lc, in0=mc, scalar=-2.0, in1=hc, op0=MULT, op1=SUB)
            hs = bhi[:, 3 * OH:4 * OH]
            ls = blo[:, 3 * OH:4 * OH]
            nc.scalar.copy(hs, sqsum[:])
            nc.vector.scalar_tensor_tensor(
                out=ls, in0=sqsum[:], scalar=1.0, in1=hs, op0=MULT, op1=SUB)

            # ---------------- flatten via DRAM roundtrip + pad ---------------
            dhi = dpool.tile([4, N_MESH], f16)
            dlo = dpool.tile([4, N_MESH], f16)
            nc.sync.dma_start(
                out=dhi[:].rearrange("c (h w) -> h c w", h=OH),
                in_=bhi[:].rearrange("h (c w) -> h c w", c=4))
            nc.sync.dma_start(
                out=dlo[:].rearrange("c (h w) -> h c w", h=OH),
                in_=blo[:].rearrange("h (c w) -> h c w", c=4))
            nc.sync.dma_start(out=baug[0:4, 0:N_MESH], in_=dhi[:])
            nc.sync.dma_start(out=baug[4:8, 0:N_MESH], in_=dlo[:])
            nc.sync.dma_start(out=baug[8:12, 0:N_MESH], in_=dhi[:])

            # ---------------- main loop ----------------
            results = cpool.tile([128, PC_TILES], f32)
            trash_v = cpool.tile([128, 1024], f16)
            trash_a = cpool.tile([128, max(a_cols, 1)], f16)
            accs_v = cpool.tile([128, PC_TILES * gv], f32)

            with tc.tile_pool(name="mpsum", bufs=cfg["psum_bufs"], space="PSUM") as mpool, \
                 tc.tile_pool(name="evac", bufs=cfg["ev_bufs"]) as epool, \
                 tc.tile_pool(name="fevac", bufs=cfg["fev_bufs"]) as fpool:
                fh = 0            # global flip-half counter
                fev = None
                fq_cur = None

                def emit_flip_half():
                    nonlocal fh, fev
                    j, h = fh // 2, fh % 2
                    mb = flip0 + j * 128
                    if h == 0:
                        fev = fpool.tile([128, M_CORE], f16, name="fev")
                    fq = mpool.tile([128, 1024], f32, name="pd")
                    for s in range(2):
                        nc.tensor.matmul(
                            out=fq[:, s * 512:(s + 1) * 512],
                            lhsT=baug[:, mb:mb + 128],
                            rhs=aaug[:, h * 1024 + s * 512:h * 1024 + (s + 1) * 512],
                            start=True, stop=True)
                    nc.scalar.mul(fev[:, h * 1024:(h + 1) * 1024], fq[:], -1.0)
                    if h == 1:
                        fpart = fpool.tile([1, M_CORE], f16, name="fpart")
                        nc.gpsimd.tensor_reduce(
                            fpart[:], fev[:], axis=mybir.AxisListType.C, op=MAX)
                        nc.sync.dma_start(out=out_flip[j:j + 1, :], in_=fpart[:])
                    fh += 1

                for t in range(PC_TILES):
                    tquads = quads
                    lh = aaug[:, t * 128:(t + 1) * 128]
                    ev = epool.tile([128, max(a_cols, 1)], f16, name="ev")
                    ev_fill = 0
                    jv = 0
                    ja = 0
                    todo_f = fhalves[t]
                    nq = len(quads)
                    c0 = 0

                    for qi, (w, r) in enumerate(tquads):
                        pd = mpool.tile([128, 1024], f32, name="pd")
                        fd = 0
                        while fd < w:
                            cw = min(512, w - fd)
                            nc.tensor.matmul(
                                out=pd[:, fd:fd + cw], lhsT=lh,
                                rhs=baug[:, c0 + fd:c0 + fd + cw],
                                start=True, stop=True)
                            fd += cw
                        if r == "V":
                            col = t * gv + jv
                            nc.vector.tensor_scalar(
                                out=trash_v[:, 0:w], in0=pd[:, 0:w],
                                scalar1=1e30, scalar2=None, op0=MIN, op1=MIN,
                                accum_out=accs_v[:, col:col + 1])
                            jv += 1
                        else:
                            nc.scalar.copy(ev[:, ev_fill:ev_fill + w], pd[:, 0:w])
                            ev_fill += w
                        c0 += w
                        # interleave flip halves across the tile
                        want = (todo_f * (qi + 1)) // nq
                        while fh - (sum(fhalves[:t])) < want:
                            emit_flip_half()
                    while fh < sum(fhalves[:t + 1]):
                        emit_flip_half()
                    if a_cols:
                        nc.vector.tensor_scalar(
                            out=trash_a[:, 0:ev_fill], in0=ev[:, 0:ev_fill],
                            scalar1=1e30, scalar2=None, op0=MIN, op1=MIN,
                            accum_out=accs_v[:, t * gv + nV:t * gv + nV + 1])
                    if t == PC_TILES // 2 - 1:
                        nc.vector.tensor_reduce(
                            results[:, 0:PC_TILES // 2],
                            accs_v[:, 0:gv * PC_TILES // 2].rearrange(
                                "p (t g) -> p t g", t=PC_TILES // 2),
                            axis=mybir.AxisListType.X, op=MIN)
                        nc.sync.dma_start(out=out_min[:, 0:PC_TILES // 2],
                                          in_=results[:, 0:PC_TILES // 2])

                nc.vector.tensor_reduce(
                    results[:, PC_TILES // 2:],
                    accs_v[:, gv * PC_TILES // 2:].rearrange(
                        "p (t g) -> p t g", t=PC_TILES // 2),
                    axis=mybir.AxisListType.X, op=MIN)

            nc.sync.dma_start(out=out_min[:, PC_TILES // 2:],
                              in_=results[:, PC_TILES // 2:])

    nc.compile()
    return nc


def _get_nc():
    if "nc" not in _BUILT:
        _BUILT["nc"] = _build_kernel()
    return _BUILT["nc"]


def _make_a_aug(pc_slice: np.ndarray) -> np.ndarray:
    m = pc_slice.shape[1]
    hi = pc_slice.astype(np.float16)
    lo = (pc_slice - hi.astype(np.float32)).astype(np.float16)
    a = np.zeros((12, m), dtype=np.float16)
    a[0:3] = hi
    a[3] = np.float16(1.0)
    a[4:7] = hi
    a[7] = np.float16(1.0)
    a[8:11] = lo
    a[11] = np.float16(0.0)
    return a


def kernel(network_mesh: np.ndarray, pc: np.ndarray) -> np.ndarray:
    global LAST_RESULTS
    from concourse.bass_utils import run_bass_kernel_spmd

    network_mesh = np.ascontiguousarray(network_mesh, dtype=np.float32)
    pc = np.ascontiguousarray(pc, dtype=np.float32)

    nc = _get_nc()
    rmat3 = np.tile(_interp_matrix().T, (C, 1))          # [96, 94]

    in_maps = []
    for core in range(N_CORES):
        b, h = core // 2, core % 2
        pre = np.zeros((C * H, OH + C * W), dtype=np.float32)
        pre[:, 0:OH] = rmat3
        pre[0:H, OH:] = network_mesh[b].transpose(1, 0, 2).reshape(H, C * W)
        in_maps.append({
            "pre": pre,
            "a_aug": _make_a_aug(pc[b, :, h * M_CORE:(h + 1) * M_CORE]),
            "padv": _PADV,
        })

    res = run_bass_kernel_spmd(nc, in_maps, core_ids=list(range(N_CORES)))
    LAST_RESULTS = res

    pnorm = np.sum(pc * pc, axis=1)
    vals = []
    for core in range(N_CORES):
        b, h = core // 2, core % 2
        v_norm = res.results[core]["minaug"].T.reshape(M_CORE)
        v_flip = -res.results[core]["flipmax"].astype(np.float32).max(axis=0)
        v = np.minimum(v_norm, v_flip)
        vals.append(v + pnorm[b, h * M_CORE:(h + 1) * M_CORE])
    dist2 = np.concatenate(vals)
    return np.array(np.mean(dist2, dtype=np.float32), dtype=np.float32)
